# revision 1
# baseline (speedup 1.0000x reference)
"""Trainium2 Bass kernel: single-head causal self-attention.

Math (torch Linear convention):
    q = x @ Wq.T ; k = x @ Wk.T ; v = x @ Wv.T          (x: [B,S,D])
    out = softmax(causal_mask(q k^T / sqrt(D))) @ v

Sharding: pure data parallel -- batch dim (32) split across 8 NeuronCores
(4 batches per core); the three 64x64 weights are replicated.

Per-core kernel (data stored fp32-compatible float32r so PE matmuls run at
1 cycle/row; PSUM accumulation is fp32):
  - X tile [128,64] loaded contiguously, PE-transposed to XT [64, S].
  - Q,K projections packed into one M=128 matmul (lhsT = [WqT|WkT]);
    softmax 1/sqrt(D) folded into WqT.
  - V projection in natural [s, h] layout, plus an appended ones column so
    the P@V matmul's row 64 yields the softmax denominator for free.
  - Scores computed TRANSPOSED (ST[k, q]) per 128-row k-tile, only for the
    causal q-range (chunks widened to >=256 cols so fp32r runs 1 cyc/row).
  - exp on ScalarE directly from PSUM; masked (q<k) region zeroed post-exp
    with affine_select on GPSIMD (scores are tiny, exp can't overflow, and
    softmax is shift-invariant so no max-subtraction pass is needed).
  - OT[h,q] accumulated over k-tiles in PSUM via has_written accumulation.
  - PE un-transpose [65,128] blocks -> [128,65] in plain fp32; col 64 is the
    denominator; reciprocal + broadcast multiply normalizes; contiguous DMA.
"""

import sys

sys.path.insert(0, "/opt/trn_rl_repo")

import numpy as np

import concourse.bass as bass
import concourse.mybir as mybir
import concourse.tile as tile
from concourse import bacc
from concourse.bass_utils import run_bass_kernel_spmd
from concourse.masks import make_identity

N_CORES = 8
B_TOTAL = 32
B = B_TOTAL // N_CORES  # batches per core
S = 1024
D = 64
NT = S // 128  # 8 row-tiles of 128
F32 = mybir.dt.float32
F32R = mybir.dt.float32r


def _chunks_for(j):
    """Causal q-range chunks [(a,b)...] for k-tile j, split at the PSUM bank
    boundary (512 f32) and widened to >=256 cols so fp32r matmuls run at
    1 cycle/row. Widened columns land in the masked q<k region."""
    q0 = j * 128
    if q0 < 512:
        a = q0 if 512 - q0 >= 256 else 512 - 256
        return a, [(a, 512), (512, 1024)]
    a = q0 if 1024 - q0 >= 256 else 1024 - 256
    return a, [(a, 1024)]


def build_bass():
    nc = bacc.Bacc("TRN2", debug=False, num_devices=N_CORES)
    x = nc.dram_tensor("x", [B, S, D], F32R, kind="ExternalInput").ap()
    wq = nc.dram_tensor("wq", [D, D], F32R, kind="ExternalInput").ap()
    wk = nc.dram_tensor("wk", [D, D], F32R, kind="ExternalInput").ap()
    wv = nc.dram_tensor("wv", [D, D], F32R, kind="ExternalInput").ap()
    out = nc.dram_tensor("out", [B, S, D], F32, kind="ExternalOutput").ap()

    with tile.TileContext(nc) as tc:
        with (
            tc.tile_pool(name="consts", bufs=1) as consts,
            tc.tile_pool(name="xp", bufs=2) as xpool,
            tc.tile_pool(name="xtp", bufs=2) as xtpool,
            tc.tile_pool(name="qtp", bufs=2) as qtpool,
            tc.tile_pool(name="ktp", bufs=2) as ktpool,
            tc.tile_pool(name="vp", bufs=2) as vpool,
            tc.tile_pool(name="ptp", bufs=3) as ptpool,
            tc.tile_pool(name="otsp", bufs=2) as otsbpool,
            tc.tile_pool(name="op", bufs=2) as opool,
            tc.tile_pool(name="rp", bufs=2) as rpool,
            tc.tile_pool(name="ps", bufs=3, space="PSUM") as pspool,
            tc.tile_pool(name="otps", bufs=1, space="PSUM") as otpool,
        ):
            identity_f = consts.tile([128, 128], F32)
            make_identity(nc, identity_f)
            identity = consts.tile([128, 128], F32R)
            nc.vector.tensor_copy(out=identity, in_=identity_f)
            wqk = consts.tile([64, 128], F32R)
            nc.sync.dma_start(out=wqk[:, 0:64], in_=wq.rearrange("h d -> d h"))
            nc.sync.dma_start(out=wqk[:, 64:128], in_=wk.rearrange("h d -> d h"))
            # fold the softmax 1/sqrt(D) scale into the Q projection weights
            nc.scalar.mul(out=wqk[:, 0:64], in_=wqk[:, 0:64], mul=D**-0.5)
            wvt = consts.tile([64, 64], F32R)
            nc.sync.dma_start(out=wvt, in_=wv.rearrange("h d -> d h"))

            for b in range(B):
                # ---- load X contiguously, PE-transpose to XT [d, s] ----
                xsb = xpool.tile([128, NT, D], F32R, tag="x")
                nc.sync.dma_start(
                    out=xsb, in_=x[b].rearrange("(so p) d -> p so d", p=128)
                )
                xt_ps = pspool.tile([64, S], F32R, tag="ps")
                for so in range(NT):
                    nc.tensor.matmul(
                        out=xt_ps[:, so * 128 : (so + 1) * 128],
                        lhsT=xsb[:, so, :],
                        rhs=identity,
                        is_transpose=True,
                    )
                xt = xtpool.tile([64, S], F32R, tag="xt")
                nc.vector.tensor_copy(out=xt, in_=xt_ps)

                # ---- Q,K projections packed into one M=128 matmul ----
                qk_ps = pspool.tile([128, S], F32, tag="ps")
                for c in range(2):
                    nc.tensor.matmul(
                        out=qk_ps[:, c * 512 : (c + 1) * 512],
                        lhsT=wqk,
                        rhs=xt[:, c * 512 : (c + 1) * 512],
                    )
                qt = qtpool.tile([64, S], F32R, tag="qt")
                kt = ktpool.tile([64, S], F32R, tag="kt")
                nc.vector.tensor_copy(out=qt, in_=qk_ps[0:64, :])
                nc.vector.tensor_copy(out=kt, in_=qk_ps[64:128, :])

                # ---- V projection in [s, h] layout + ones column ----
                v_ps = pspool.tile([128, NT * D], F32, tag="ps")
                for so in range(NT):
                    nc.tensor.matmul(
                        out=v_ps[:, so * D : (so + 1) * D],
                        lhsT=xt[:, so * 128 : (so + 1) * 128],
                        rhs=wvt,
                    )
                vsb = vpool.tile([128, NT, D + 1], F32R, tag="v")
                # contiguous f32 memset sets the ones column; V-copy overwrites data
                nc.vector.memset(vsb.bitcast(F32), 1.0)
                nc.vector.tensor_copy(
                    out=vsb[:, :, 0:D], in_=v_ps.rearrange("p (so d) -> p so d", d=D)
                )

                # ---- k-tile loop: ST = (K_j @ QT), exp, mask, OT += V_j^T @ P ----
                ot = otpool.tile([65, S], F32, tag="ot")
                for j in range(NT):
                    sa, chs = _chunks_for(j)
                    w = S - sa
                    st = pspool.tile([128, S], F32, tag="ps")
                    for ca, cb in chs:
                        nc.tensor.matmul(
                            out=st[:, ca:cb],
                            lhsT=kt[:, j * 128 : (j + 1) * 128],
                            rhs=qt[:, ca:cb],
                        )
                    pt = ptpool.tile([128, S], F32R, tag="pt")
                    nc.scalar.activation(
                        out=pt[:, 0:w],
                        in_=st[:, sa:S],
                        func=mybir.ActivationFunctionType.Exp,
                    )
                    # zero the masked q<k region: pt cols [0, (j+1)*128 - sa)
                    mw = (j + 1) * 128 - sa
                    nc.gpsimd.affine_select(
                        out=pt[:, 0:mw],
                        in_=pt[:, 0:mw],
                        compare_op=mybir.AluOpType.is_ge,
                        fill=0.0,
                        base=sa - j * 128,
                        pattern=[[1, mw]],
                        channel_multiplier=-1,
                    )
                    for ca, cb in chs:
                        bank = 0 if ca < 512 else 1
                        nc.tensor.matmul(
                            out=ot[:, ca:cb],
                            lhsT=vsb[:, j, :],
                            rhs=pt[:, ca - sa : cb - sa],
                            start=(j == 0),
                            stop=(j == 3 and bank == 0) or (j == 7 and bank == 1),
                            skip_group_check=True,
                        )

                # ---- un-transpose, normalize by row 64 (denominator), store ----
                otsb = otsbpool.tile([65, S], F32, tag="otsb")
                nc.vector.tensor_copy(out=otsb, in_=ot)
                osb = opool.tile([128, NT, D], F32, tag="o")
                rsb = rpool.tile([128, NT], F32, tag="r")
                id65 = identity[0:65, 0:65].bitcast(F32)
                for half in range(2):
                    otr = pspool.tile([128, 4, D + 1], F32, tag="ps")
                    for t in range(4):
                        i = half * 4 + t
                        nc.tensor.matmul(
                            out=otr[:, t, :],
                            lhsT=otsb[:, i * 128 : (i + 1) * 128],
                            rhs=id65,
                            is_transpose=True,
                        )
                    rs = rsb[:, half * 4 : (half + 1) * 4]
                    nc.vector.reciprocal(out=rs, in_=otr[:, :, D])
                    r_bc = bass.AP(
                        tensor=rs.tensor,
                        offset=rs.offset,
                        ap=[rs.ap[0], rs.ap[1], [0, D]],
                    )
                    nc.vector.tensor_mul(
                        out=osb[:, half * 4 : (half + 1) * 4, :],
                        in0=otr[:, :, 0:D],
                        in1=r_bc,
                    )
                nc.sync.dma_start(
                    out=out[b].rearrange("(so p) d -> p so d", p=128), in_=osb
                )
    # bacc lowering: moves matmul waits onto LDWEIGHTS, converts multi-wait
    # nops/drains to events, allocates registers -- required for walrus codegen
    nc.compile()
    return nc


_NC_CACHE = []
LAST_RESULTS = None


def kernel(x, Wq, Wk, Wv):
    global LAST_RESULTS
    if not _NC_CACHE:
        _NC_CACHE.append(build_bass())
    nc = _NC_CACHE[0]
    x = np.ascontiguousarray(x, dtype=np.float32)
    in_maps = [
        {
            "x": np.ascontiguousarray(x[c * B : (c + 1) * B]),
            "wq": np.ascontiguousarray(Wq, dtype=np.float32),
            "wk": np.ascontiguousarray(Wk, dtype=np.float32),
            "wv": np.ascontiguousarray(Wv, dtype=np.float32),
        }
        for c in range(N_CORES)
    ]
    res = run_bass_kernel_spmd(nc, in_maps, core_ids=list(range(N_CORES)))
    LAST_RESULTS = res
    return np.concatenate([r["out"] for r in res.results], axis=0)



# revision 5
# speedup vs baseline: 1.5018x; 1.5018x over previous
"""Trainium2 Bass kernel: single-head causal self-attention (v2b).

Math (torch Linear convention):
    q = x @ Wq.T ; k = x @ Wk.T ; v = x @ Wv.T          (x: [B,S,D])
    out = softmax(causal_mask(q k^T / sqrt(D))) @ v

Key identity: scores = q k^T / 8 = x G x^T with G = (Wq^T Wk)/8, so only ONE
projection (UT = G^T x^T) is needed instead of Q and K.  The P-accumulation
matmul uses a [V|1|0] stationary: the ones column yields the softmax
denominator, the zero rows pad partitions to 80 for the XBAR un-transpose.

Sharding: pure data parallel -- batch dim (32) split across 8 NeuronCores
(4 batches per core); weights replicated.

Per-core structure (big matmuls bf16; PSUM accum fp32):
  - x loaded f32, cast bf16 into a 2-batch packed [128, (so i d)] tile;
    one XBAR DMA-transpose produces XT2 [128=(i d), S] bf16 (no PE transpose).
  - UT2 = blockdiag(G,G) @ XT2 (full 128-contraction, both batches at once).
  - V = XT_chunk^T @ WvT per 128-row tile, cast bf16 into the persistent
    [V|1|0] lhsT tile.
  - Scores TRANSPOSED (ST[k,q]) per k-tile, exact causal q-range, bf16.
  - exp on ScalarE from PSUM straight to bf16 (scores are tiny: no max pass).
  - causal mask on the 128-wide diagonal block only (GpSimd affine_select).
  - OT[80, S] accumulated over k-tiles in PSUM.
  - XBAR DMA-transpose back to natural layout; reciprocal of the denominator
    column + broadcast multiply normalizes; contiguous DMA out.
"""

import sys

sys.path.insert(0, "/opt/trn_rl_repo")

import numpy as np

import concourse.bass as bass
import concourse.mybir as mybir
import concourse.tile as tile
from concourse import bacc
from concourse.bass_utils import run_bass_kernel_spmd
from concourse.masks import make_identity

N_CORES = 8
B_TOTAL = 32
B = B_TOTAL // N_CORES  # batches per core
S = 1024
D = 64
NT = S // 128  # 8 k-tiles of 128
F32 = mybir.dt.float32
BF16 = mybir.dt.bfloat16


def _chunks(a, b):
    """Split [a, b) at the 512 PSUM-bank boundary."""
    out = []
    if a < 512:
        out.append((a, min(512, b)))
    if b > 512:
        out.append((max(a, 512), b))
    return out


def build_bass():
    nc = bacc.Bacc("TRN2", debug=False, num_devices=N_CORES)
    x = nc.dram_tensor("x", [B, S, D], F32, kind="ExternalInput").ap()
    wq = nc.dram_tensor("wq", [D, D], F32, kind="ExternalInput").ap()
    wk = nc.dram_tensor("wk", [D, D], F32, kind="ExternalInput").ap()
    wv = nc.dram_tensor("wv", [D, D], F32, kind="ExternalInput").ap()
    out = nc.dram_tensor("out", [B, S, D], F32, kind="ExternalOutput").ap()

    Exp = mybir.ActivationFunctionType.Exp

    with tile.TileContext(nc) as tc:
        with (
            tc.tile_pool(name="consts", bufs=1) as consts,
            tc.tile_pool(name="xf", bufs=4) as xfp,
            tc.tile_pool(name="x2", bufs=2) as x2p,
            tc.tile_pool(name="xt", bufs=2) as xtp,
            tc.tile_pool(name="ut", bufs=2) as utp,
            tc.tile_pool(name="pt", bufs=3) as ptp,
            tc.tile_pool(name="osb", bufs=2) as osbp,
            tc.tile_pool(name="on", bufs=2) as onp,
            tc.tile_pool(name="ob", bufs=2) as obp,
            tc.tile_pool(name="rp", bufs=2) as rpp,
            tc.tile_pool(name="pj", bufs=2, space="PSUM") as pjp,
            tc.tile_pool(name="st", bufs=2, space="PSUM") as stp,
            tc.tile_pool(name="ot", bufs=2, space="PSUM") as otp,
        ):
            # ---------------- setup ----------------
            id64 = consts.tile([64, 64], F32)
            make_identity(nc, id64)
            wqs = consts.tile([64, 64], F32)
            nc.sync.dma_start(out=wqs, in_=wq)
            wks = consts.tile([64, 64], F32)
            nc.sync.dma_start(out=wks, in_=wk)
            wvs = consts.tile([64, 64], F32)
            nc.sync.dma_start(out=wvs, in_=wv)

            # gt[d,d'] = sum_h Wq[h,d] Wk[h,d']; scores = x (gt/8) x^T
            gt_ps = pjp.tile([64, 64], F32, tag="pj")
            nc.tensor.matmul(out=gt_ps, lhsT=wqs, rhs=wks)
            g2 = consts.tile([128, 128], BF16)
            nc.vector.memset(g2, 0.0)
            nc.scalar.mul(out=g2[0:64, 0:64], in_=gt_ps, mul=D**-0.5)
            nc.scalar.mul(out=g2[64:128, 64:128], in_=gt_ps, mul=D**-0.5)

            # WvT duplicated on both partition halves (moving operand for V)
            wvt_ps = pjp.tile([64, 64], F32, tag="pj")
            nc.tensor.matmul(out=wvt_ps, lhsT=wvs, rhs=id64, is_transpose=True)
            wvt2 = consts.tile([128, 64], BF16)
            nc.vector.tensor_copy(out=wvt2[0:64, :], in_=wvt_ps)
            nc.vector.tensor_copy(out=wvt2[64:128, :], in_=wvt_ps)

            # persistent [V|1|0] bf16 lhsT: slot (b, j) at [:, b*NT + j, :]
            vq = consts.tile([128, B * NT, 80], BF16)
            nc.gpsimd.memset(vq[:, :, 64:65], 1.0)
            nc.gpsimd.memset(vq[:, :, 65:80], 0.0)

            # ---------------- x loads (all upfront) ----------------
            xs = []
            for b in range(B):
                xb = xfp.tile([128, NT, D], F32, tag="x", name=f"xs{b}")
                nc.sync.dma_start(
                    out=xb, in_=x[b].rearrange("(so p) d -> p so d", p=128)
                )
                xs.append(xb)

            def prep_pair(p):
                """bf16 pack + XBAR transpose + UT2 + V for batches 2p, 2p+1."""
                b0 = 2 * p
                x2bf = x2p.tile([128, NT, 2, D], BF16, tag="x2", name=f"x2bf{p}")
                nc.vector.tensor_copy(out=x2bf[:, :, 0, :], in_=xs[b0])
                nc.vector.tensor_copy(out=x2bf[:, :, 1, :], in_=xs[b0 + 1])
                xt2 = xtp.tile([128, NT, 128], BF16, tag="xt", name=f"xt2{p}")
                nc.sync.dma_start_transpose(out=xt2, in_=x2bf)
                ut_ps = []
                for h in range(2):
                    u = pjp.tile([128, 512], F32, tag="pj", name=f"utps{p}{h}")
                    nc.tensor.matmul(
                        out=u, lhsT=g2, rhs=xt2[:, 4 * h : 4 * h + 4, :]
                    )
                    ut_ps.append(u)
                ut2 = utp.tile([128, 2, 512], BF16, tag="ut", name=f"ut2{p}")
                nc.vector.tensor_copy(out=ut2[:, 0, :], in_=ut_ps[0])
                nc.vector.tensor_copy(out=ut2[:, 1, :], in_=ut_ps[1])
                for i in range(2):
                    b = b0 + i
                    r0 = 64 * i
                    v_ps = pjp.tile([128, NT, D], F32, tag="pj", name=f"vps{b}")
                    for so in range(NT):
                        nc.tensor.matmul(
                            out=v_ps[:, so, :],
                            lhsT=xt2[r0 : r0 + 64, so, :],
                            rhs=wvt2[r0 : r0 + 64, :],
                        )
                    nc.vector.tensor_copy(
                        out=vq[:, b * NT : (b + 1) * NT, 0:64], in_=v_ps
                    )
                return xt2, ut2

            def attn(b, i, xt2, ut2):
                """Causal attention for one batch (partition half i of pair)."""
                r0 = 64 * i
                ot_h = [
                    otp.tile([80, 512], F32, tag="ot", name=f"ot{b}{h}")
                    for h in range(2)
                ]
                for j in range(NT):
                    qa = j * 128
                    st = stp.tile([128, 1024], F32, tag="st", name=f"st{b}{j}")
                    for ca, cb in _chunks(qa, S):
                        nc.tensor.matmul(
                            out=st[:, ca:cb],
                            lhsT=xt2[r0 : r0 + 64, j, :],
                            rhs=ut2[
                                r0 : r0 + 64, ca // 512, ca % 512 : ca % 512 + cb - ca
                            ],
                        )
                    # pt local col c <-> q = qa + c
                    pt = ptp.tile([128, 1024], BF16, tag="pt", name=f"pt{b}{j}")
                    nc.scalar.activation(
                        out=pt[:, 0 : S - qa], in_=st[:, qa:S], func=Exp
                    )
                    # triangular causal mask on the diagonal block
                    nc.gpsimd.affine_select(
                        out=pt[:, 0:128],
                        in_=pt[:, 0:128],
                        compare_op=mybir.AluOpType.is_ge,
                        fill=0.0,
                        base=0,
                        pattern=[[1, 128]],
                        channel_multiplier=-1,
                    )
                    for ca, cb in _chunks(qa, S):
                        h = ca // 512
                        nc.tensor.matmul(
                            out=ot_h[h][:, ca - 512 * h : cb - 512 * h],
                            lhsT=vq[:, b * NT + j, :],
                            rhs=pt[:, ca - qa : cb - qa],
                            start=(j == 0),
                            stop=(j == 3 if h == 0 else j == 7),
                            skip_group_check=True,
                        )
                otsb = osbp.tile([80, 1024], BF16, tag="osb", name=f"otsb{b}")
                nc.vector.tensor_copy(out=otsb[:, 0:512], in_=ot_h[0])
                nc.vector.tensor_copy(out=otsb[:, 512:1024], in_=ot_h[1])
                ot_nat = onp.tile([128, NT, 80], BF16, tag="on", name=f"onat{b}")
                nc.sync.dma_start_transpose(out=ot_nat, in_=otsb)
                rsb = rpp.tile([128, NT], F32, tag="r", name=f"rsb{b}")
                nc.vector.reciprocal(out=rsb, in_=ot_nat[:, :, 64])
                r_bc = bass.AP(
                    tensor=rsb.tensor,
                    offset=rsb.offset,
                    ap=[rsb.ap[0], rsb.ap[1], [0, D]],
                )
                osb = obp.tile([128, NT, D], F32, tag="ob", name=f"osb{b}")
                nc.vector.tensor_mul(out=osb, in0=ot_nat[:, :, 0:64], in1=r_bc)
                nc.sync.dma_start(
                    out=out[b].rearrange("(so p) d -> p so d", p=128), in_=osb
                )

            xt2_0, ut2_0 = prep_pair(0)
            attn(0, 0, xt2_0, ut2_0)
            xt2_1, ut2_1 = prep_pair(1)
            attn(1, 1, xt2_0, ut2_0)
            attn(2, 0, xt2_1, ut2_1)
            attn(3, 1, xt2_1, ut2_1)
    nc.compile()
    return nc


_NC_CACHE = []
LAST_RESULTS = None


def kernel(x, Wq, Wk, Wv):
    global LAST_RESULTS
    if not _NC_CACHE:
        _NC_CACHE.append(build_bass())
    nc = _NC_CACHE[0]
    x = np.ascontiguousarray(x, dtype=np.float32)
    in_maps = [
        {
            "x": np.ascontiguousarray(x[c * B : (c + 1) * B]),
            "wq": np.ascontiguousarray(Wq, dtype=np.float32),
            "wk": np.ascontiguousarray(Wk, dtype=np.float32),
            "wv": np.ascontiguousarray(Wv, dtype=np.float32),
        }
        for c in range(N_CORES)
    ]
    res = run_bass_kernel_spmd(nc, in_maps, core_ids=list(range(N_CORES)))
    LAST_RESULTS = res
    return np.concatenate([r["out"] for r in res.results], axis=0)


# revision 11
# speedup vs baseline: 1.5787x; 1.0512x over previous
"""Trainium2 Bass kernel: single-head causal self-attention (v3).

Math (torch Linear convention):
    q = x @ Wq.T ; k = x @ Wk.T ; v = x @ Wv.T          (x: [B,S,D])
    out = softmax(causal_mask(q k^T / sqrt(D))) @ v

Key identity: scores = q k^T / 8 = x G x^T with G = (Wq^T Wk)/8, so only ONE
projection (UT = G^T x^T) is needed instead of Q and K.  The P-accumulation
matmul uses a [V|1] stationary: the ones column yields the softmax denominator.

Score matmuls run in fp8e4m3 DoubleRow mode (2 cols/cycle) with a zero-padded
second K-slot; x~N(0,1) and UT (rescaled x64 to clear the e4m3 subnormal
range, folded back via the exp activation scale) quantize safely because the
64-deep contraction averages the 6% element error down to ~1%.  P and V stay
bf16 (P fp8 alone costs ~2.6e-2 rel err -- over the 2e-2 gate).

Sharding: pure data parallel -- batch dim (32) split across 8 NeuronCores
(4 batches per core); weights replicated.

Startup/tail: the four x loads dispatch from four different engines (DGE
dispatch is ~1.3us per DMA, serializing on one queue otherwise); batches 0-2
un-transpose via XBAR DMA (off the critical path), batch 3 via PE transposes
(avoids two ~1us DMA-completion semaphore hops on the exposed tail).
"""

import sys

sys.path.insert(0, "/opt/trn_rl_repo")

import numpy as np

import concourse.bass as bass
import concourse.mybir as mybir
import concourse.tile as tile
from concourse import bacc
from concourse.bass_utils import run_bass_kernel_spmd
from concourse.masks import make_identity

N_CORES = 8
B_TOTAL = 32
B = B_TOTAL // N_CORES  # batches per core
S = 1024
D = 64
NT = S // 128  # 8 k-tiles of 128
F32 = mybir.dt.float32
BF16 = mybir.dt.bfloat16
FP8 = mybir.dt.float8e4
USCALE = 64.0  # fp8 headroom scale on UT, undone by exp's activation scale


def _chunks(a, b):
    """Split [a, b) at the 512 PSUM-bank boundary."""
    out = []
    if a < 512:
        out.append((a, min(512, b)))
    if b > 512:
        out.append((max(a, 512), b))
    return out


def build_bass():
    nc = bacc.Bacc("TRN2", debug=False, num_devices=N_CORES)
    x = nc.dram_tensor("x", [B, S, D], F32, kind="ExternalInput").ap()
    wq = nc.dram_tensor("wq", [D, D], F32, kind="ExternalInput").ap()
    wk = nc.dram_tensor("wk", [D, D], F32, kind="ExternalInput").ap()
    wv = nc.dram_tensor("wv", [D, D], F32, kind="ExternalInput").ap()
    out = nc.dram_tensor("out", [B, S, D], F32, kind="ExternalOutput").ap()

    Exp = mybir.ActivationFunctionType.Exp
    DR = mybir.MatmulPerfMode.DoubleRow

    with tile.TileContext(nc) as tc:
        with (
            tc.tile_pool(name="consts", bufs=1) as consts,
            tc.tile_pool(name="xf", bufs=4) as xfp,
            tc.tile_pool(name="x2", bufs=2) as x2p,
            tc.tile_pool(name="xt", bufs=2) as xtp,
            tc.tile_pool(name="pt", bufs=3) as ptp,
            tc.tile_pool(name="osb", bufs=2) as osbp,
            tc.tile_pool(name="on", bufs=2) as onp,
            tc.tile_pool(name="ob", bufs=2) as obp,
            tc.tile_pool(name="rp", bufs=2) as rpp,
            tc.tile_pool(name="pj", bufs=2, space="PSUM") as pjp,
            tc.tile_pool(name="st", bufs=2, space="PSUM") as stp,
            tc.tile_pool(name="ot", bufs=2, space="PSUM") as otp,
        ):
            # ------------- weight loads (gpsimd queue; x loads elsewhere) ----
            wqs = consts.tile([64, 64], F32)
            nc.gpsimd.dma_start(out=wqs, in_=wq)
            wks = consts.tile([64, 64], F32)
            nc.gpsimd.dma_start(out=wks, in_=wk)
            wvs = consts.tile([64, 64], F32)
            nc.gpsimd.dma_start(out=wvs, in_=wv)

            # ------------- x loads: one per engine, parallel dispatch -------
            xs = []
            for b, eng in enumerate((nc.sync, nc.scalar, nc.gpsimd, nc.sync)):
                xb = xfp.tile([128, NT, D], F32, tag="x", name=f"xs{b}")
                eng.dma_start(
                    out=xb, in_=x[b].rearrange("(so p) d -> p so d", p=128)
                )
                xs.append(xb)

            # ------------- setup ----------------
            id64 = consts.tile([64, 64], F32)
            make_identity(nc, id64)
            idb = consts.tile([65, 65], BF16)
            nc.gpsimd.memset(idb, 0.0)
            nc.vector.tensor_copy(out=idb[0:64, 0:64], in_=id64)
            nc.gpsimd.memset(idb[64:65, 64:65], 1.0)

            # gt[d,d'] = sum_h Wq[h,d] Wk[h,d']; scores = x (gt/8) x^T
            gt_ps = pjp.tile([64, 64], F32, tag="pj")
            nc.tensor.matmul(out=gt_ps, lhsT=wqs, rhs=wks)
            g2 = consts.tile([128, 128], BF16)
            nc.vector.memset(g2, 0.0)
            gsc = USCALE * D**-0.5
            nc.scalar.mul(out=g2[0:64, 0:64], in_=gt_ps, mul=gsc)
            nc.scalar.mul(out=g2[64:128, 64:128], in_=gt_ps, mul=gsc)

            # WvT duplicated block-diagonally: V for both batches in one go
            wvt_ps = pjp.tile([64, 64], F32, tag="pj")
            nc.tensor.matmul(out=wvt_ps, lhsT=wvs, rhs=id64, is_transpose=True)
            wvt2 = consts.tile([128, 128], BF16)
            nc.vector.memset(wvt2, 0.0)
            nc.vector.tensor_copy(out=wvt2[0:64, 0:64], in_=wvt_ps)
            nc.vector.tensor_copy(out=wvt2[64:128, 64:128], in_=wvt_ps)

            # persistent [V|1] bf16 lhsT: slot (b, j) at [:, b*NT + j, :]
            vq = consts.tile([128, B * NT, 65], BF16)
            nc.gpsimd.memset(vq[:, :, 64:65], 1.0)

            # persistent fp8 ST operands; slot 1 zeroed once (DoubleRow pad)
            xq8 = consts.tile([128, 2, NT, 2, 128], FP8)
            nc.gpsimd.memset(xq8[:, :, :, 1, :], 0.0)
            ut8 = consts.tile([128, 2, 2, 2, 512], FP8)
            nc.gpsimd.memset(ut8[:, :, :, 1, :], 0.0)

            def prep_pair(p):
                """bf16 pack + XBAR transpose + UT + V for batches 2p, 2p+1."""
                b0 = 2 * p
                x2bf = x2p.tile([128, NT, 2, D], BF16, tag="x2", name=f"x2bf{p}")
                nc.vector.tensor_copy(out=x2bf[:, :, 0, :], in_=xs[b0])
                nc.vector.tensor_copy(out=x2bf[:, :, 1, :], in_=xs[b0 + 1])
                xt2 = xtp.tile([128, NT, 128], BF16, tag="xt", name=f"xt2{p}")
                nc.sync.dma_start_transpose(out=xt2, in_=x2bf)
                nc.vector.tensor_copy(out=xq8[:, p, :, 0, :], in_=xt2)
                for h in range(2):
                    u = pjp.tile([128, 512], F32, tag="pj", name=f"utps{p}{h}")
                    nc.tensor.matmul(
                        out=u, lhsT=g2, rhs=xt2[:, 4 * h : 4 * h + 4, :]
                    )
                    if h == 0:
                        nc.scalar.copy(out=ut8[:, p, h, 0, :], in_=u)
                    else:
                        nc.vector.tensor_copy(out=ut8[:, p, h, 0, :], in_=u)
                # V for both batches: blockdiag WvT, 128-col output per tile
                v_ps = stp.tile([128, NT, 128], F32, tag="st", name=f"vps{p}")
                for so in range(NT):
                    nc.tensor.matmul(
                        out=v_ps[:, so, :], lhsT=xt2[:, so, :], rhs=wvt2
                    )
                nc.vector.tensor_copy(
                    out=vq[:, b0 * NT : (b0 + 1) * NT, 0:64],
                    in_=v_ps[:, :, 0:64],
                )
                nc.vector.tensor_copy(
                    out=vq[:, (b0 + 1) * NT : (b0 + 2) * NT, 0:64],
                    in_=v_ps[:, :, 64:128],
                )
                return xt2

            def attn(b, i, p, last=False):
                """Causal attention for one batch (partition half i of pair p)."""
                r0 = 64 * i
                ot_h = [
                    otp.tile([65, 512], F32, tag="ot", name=f"ot{b}{h}")
                    for h in range(2)
                ]
                for j in range(NT):
                    qa = j * 128
                    st = stp.tile([128, 1024], F32, tag="st", name=f"st{b}{j}")
                    for ca, cb in _chunks(qa, S):
                        h = ca // 512
                        nc.tensor.matmul(
                            out=st[:, ca:cb],
                            lhsT=xq8[r0 : r0 + 64, p, j, :, :],
                            rhs=ut8[
                                r0 : r0 + 64, p, h, :, ca % 512 : ca % 512 + cb - ca
                            ],
                            perf_mode=DR,
                        )
                    # pt local col c <-> q = qa + c
                    pt = ptp.tile([128, 1024], BF16, tag="pt", name=f"pt{b}{j}")
                    nc.scalar.activation(
                        out=pt[:, 0 : S - qa],
                        in_=st[:, qa:S],
                        func=Exp,
                        scale=1.0 / USCALE,
                    )
                    # triangular causal mask on the diagonal block
                    nc.gpsimd.affine_select(
                        out=pt[:, 0:128],
                        in_=pt[:, 0:128],
                        compare_op=mybir.AluOpType.is_ge,
                        fill=0.0,
                        base=0,
                        pattern=[[1, 128]],
                        channel_multiplier=-1,
                    )
                    for ca, cb in _chunks(qa, S):
                        h = ca // 512
                        nc.tensor.matmul(
                            out=ot_h[h][:, ca - 512 * h : cb - 512 * h],
                            lhsT=vq[:, b * NT + j, :],
                            rhs=pt[:, ca - qa : cb - qa],
                            start=(j == 0),
                            stop=(j == 3 if h == 0 else j == 7),
                            skip_group_check=True,
                        )
                # [80, ...]: rows 65:80 stay unwritten junk; the XBAR reads
                # them but their transposed columns are never consumed
                otsb = osbp.tile([80, 1024], BF16, tag="osb", name=f"otsb{b}")
                nc.vector.tensor_copy(out=otsb[0:65, 0:512], in_=ot_h[0])
                nc.vector.tensor_copy(out=otsb[0:65, 512:1024], in_=ot_h[1])
                if not last:
                    # off-critical-path batches: XBAR DMA un-transpose
                    ot_nat = onp.tile(
                        [128, NT, 80], BF16, tag="on", name=f"onat{b}"
                    )
                    nc.sync.dma_start_transpose(out=ot_nat, in_=otsb)
                    src = ot_nat
                else:
                    # last batch: PE transposes avoid DMA-semaphore tail hops
                    # (inner dim padded to 66 to keep PSUM writes 4B-aligned)
                    ot_tp = pjp.tile([128, NT, 66], BF16, tag="pj", name="ottp")
                    for so in range(NT):
                        nc.tensor.matmul(
                            out=ot_tp[:, so, 0:65],
                            lhsT=otsb[0:65, so * 128 : (so + 1) * 128],
                            rhs=idb,
                            is_transpose=True,
                        )
                    src = ot_tp
                rsb = rpp.tile([128, NT], F32, tag="r", name=f"rsb{b}")
                nc.vector.reciprocal(out=rsb, in_=src[:, :, 64])
                r_bc = bass.AP(
                    tensor=rsb.tensor,
                    offset=rsb.offset,
                    ap=[rsb.ap[0], rsb.ap[1], [0, D]],
                )
                osb = obp.tile([128, NT, D], F32, tag="ob", name=f"osb{b}")
                nc.vector.tensor_mul(out=osb, in0=src[:, :, 0:64], in1=r_bc)
                nc.sync.dma_start(
                    out=out[b].rearrange("(so p) d -> p so d", p=128), in_=osb
                )

            prep_pair(0)
            attn(0, 0, 0)
            prep_pair(1)
            attn(1, 1, 0)
            attn(2, 0, 1)
            attn(3, 1, 1, last=True)
    nc.compile()
    return nc


_NC_CACHE = []
LAST_RESULTS = None


def kernel(x, Wq, Wk, Wv):
    global LAST_RESULTS
    if not _NC_CACHE:
        _NC_CACHE.append(build_bass())
    nc = _NC_CACHE[0]
    x = np.ascontiguousarray(x, dtype=np.float32)
    in_maps = [
        {
            "x": np.ascontiguousarray(x[c * B : (c + 1) * B]),
            "wq": np.ascontiguousarray(Wq, dtype=np.float32),
            "wk": np.ascontiguousarray(Wk, dtype=np.float32),
            "wv": np.ascontiguousarray(Wv, dtype=np.float32),
        }
        for c in range(N_CORES)
    ]
    res = run_bass_kernel_spmd(nc, in_maps, core_ids=list(range(N_CORES)))
    LAST_RESULTS = res
    return np.concatenate([r["out"] for r in res.results], axis=0)


# revision 14
# speedup vs baseline: 1.6738x; 1.0602x over previous
"""Trainium2 Bass kernel: single-head causal self-attention (v4).

Math (torch Linear convention):
    q = x @ Wq.T ; k = x @ Wk.T ; v = x @ Wv.T          (x: [B,S,D])
    out = softmax(causal_mask(q k^T / sqrt(D))) @ v

Key identity: scores = q k^T / 8 = x G x^T with G = (Wq^T Wk)/8, so only ONE
projection (UT = G^T x^T) is needed instead of Q and K.  The P-accumulation
matmul uses a [V|1] stationary: the ones column yields the softmax denominator.

All big matmuls bf16 (measured HW streams ~1 output column/ns regardless of
dtype, so fp8 buys nothing and bf16 keeps the error budget comfortable).

Sharding: pure data parallel -- batch dim (32) split across 8 NeuronCores
(4 batches per core); weights replicated.

Per-core structure:
  - x loaded f32 (loads fan out across SP/Act/Pool DGE queues -- dispatch is
    ~1.3us per DMA and serializes per queue), cast bf16 into a 2-batch packed
    tile; one XBAR DMA-transpose gives XT2 [128=(batch,d), S] (no PE work).
  - UT2 = blockdiag(G,G) @ XT2; V for both batches via blockdiag(WvT,WvT).
  - Scores TRANSPOSED (ST[k,q]) per k-tile, exact causal q-range.
  - exp on ScalarE from PSUM straight to bf16 (scores are tiny: no max pass).
  - causal mask on the 128-wide diagonal block only (GpSimd affine_select).
  - OT[65, S] accumulated over k-tiles in PSUM.
  - batches 0-2 un-transpose via XBAR DMA (off critical path); batch 3 via PE
    transposes (avoids two ~1us DMA-semaphore hops on the exposed tail).
  - reciprocal of denominator row + broadcast multiply; contiguous DMA out.
"""

import sys

sys.path.insert(0, "/opt/trn_rl_repo")

import numpy as np

import concourse.bass as bass
import concourse.mybir as mybir
import concourse.tile as tile
from concourse import bacc
from concourse.bass_utils import run_bass_kernel_spmd
from concourse.masks import make_identity

N_CORES = 8
B_TOTAL = 32
B = B_TOTAL // N_CORES  # batches per core
S = 1024
D = 64
NT = S // 128  # 8 k-tiles of 128
F32 = mybir.dt.float32
BF16 = mybir.dt.bfloat16


def _chunks(a, b):
    """Split [a, b) at the 512 PSUM-bank boundary."""
    out = []
    if a < 512:
        out.append((a, min(512, b)))
    if b > 512:
        out.append((max(a, 512), b))
    return out


def build_bass():
    nc = bacc.Bacc("TRN2", debug=False, num_devices=N_CORES)
    x = nc.dram_tensor("x", [B, S, D], F32, kind="ExternalInput").ap()
    wq = nc.dram_tensor("wq", [D, D], F32, kind="ExternalInput").ap()
    wk = nc.dram_tensor("wk", [D, D], F32, kind="ExternalInput").ap()
    wv = nc.dram_tensor("wv", [D, D], F32, kind="ExternalInput").ap()
    out = nc.dram_tensor("out", [B, S, D], F32, kind="ExternalOutput").ap()

    Exp = mybir.ActivationFunctionType.Exp

    with tile.TileContext(nc) as tc:
        with (
            tc.tile_pool(name="consts", bufs=1) as consts,
            tc.tile_pool(name="xf", bufs=4) as xfp,
            tc.tile_pool(name="x2", bufs=2) as x2p,
            tc.tile_pool(name="xt", bufs=2) as xtp,
            tc.tile_pool(name="ut", bufs=2) as utp,
            tc.tile_pool(name="pt", bufs=3) as ptp,
            tc.tile_pool(name="osb", bufs=2) as osbp,
            tc.tile_pool(name="on", bufs=2) as onp,
            tc.tile_pool(name="ob", bufs=2) as obp,
            tc.tile_pool(name="rp", bufs=2) as rpp,
            tc.tile_pool(name="pj", bufs=2, space="PSUM") as pjp,
            tc.tile_pool(name="st", bufs=2, space="PSUM") as stp,
            tc.tile_pool(name="ot", bufs=2, space="PSUM") as otp,
        ):
            # ------------- weight loads first (sync queue) ------------------
            wqs = consts.tile([64, 64], F32)
            nc.sync.dma_start(out=wqs, in_=wq)
            wks = consts.tile([64, 64], F32)
            nc.sync.dma_start(out=wks, in_=wk)
            wvs = consts.tile([64, 64], F32)
            nc.sync.dma_start(out=wvs, in_=wv)

            # ------------- x loads: separate DGE queues, parallel dispatch --
            xs = []
            for b, eng in enumerate((nc.sync, nc.scalar, nc.gpsimd, nc.sync)):
                xb = xfp.tile([128, NT, D], F32, tag="x", name=f"xs{b}")
                eng.dma_start(
                    out=xb, in_=x[b].rearrange("(so p) d -> p so d", p=128)
                )
                xs.append(xb)

            # ------------- setup ----------------
            id64 = consts.tile([64, 64], F32)
            make_identity(nc, id64)
            idb = consts.tile([65, 65], BF16)
            nc.gpsimd.memset(idb, 0.0)
            nc.vector.tensor_copy(out=idb[0:64, 0:64], in_=id64)
            nc.gpsimd.memset(idb[64:65, 64:65], 1.0)

            # gt[d,d'] = sum_h Wq[h,d] Wk[h,d']; scores = x (gt/8) x^T
            gt_ps = pjp.tile([64, 64], F32, tag="pj")
            nc.tensor.matmul(out=gt_ps, lhsT=wqs, rhs=wks)
            g2 = consts.tile([128, 128], BF16)
            nc.vector.memset(g2, 0.0)
            nc.scalar.mul(out=g2[0:64, 0:64], in_=gt_ps, mul=D**-0.5)
            nc.scalar.mul(out=g2[64:128, 64:128], in_=gt_ps, mul=D**-0.5)

            # WvT duplicated block-diagonally: V for both batches in one go
            wvt_ps = pjp.tile([64, 64], F32, tag="pj")
            nc.tensor.matmul(out=wvt_ps, lhsT=wvs, rhs=id64, is_transpose=True)
            wvt2 = consts.tile([128, 128], BF16)
            nc.vector.memset(wvt2, 0.0)
            nc.vector.tensor_copy(out=wvt2[0:64, 0:64], in_=wvt_ps)
            nc.vector.tensor_copy(out=wvt2[64:128, 64:128], in_=wvt_ps)

            # persistent [V|1] bf16 lhsT: slot (b, j) at [:, b*NT + j, :]
            vq = consts.tile([128, B * NT, 65], BF16)
            nc.gpsimd.memset(vq[:, :, 64:65], 1.0)

            def prep_pair(p):
                """bf16 pack + XBAR transpose + UT + V for batches 2p, 2p+1."""
                b0 = 2 * p
                x2bf = x2p.tile([128, NT, 2, D], BF16, tag="x2", name=f"x2bf{p}")
                nc.vector.tensor_copy(out=x2bf[:, :, 0, :], in_=xs[b0])
                nc.vector.tensor_copy(out=x2bf[:, :, 1, :], in_=xs[b0 + 1])
                xt2 = xtp.tile([128, NT, 128], BF16, tag="xt", name=f"xt2{p}")
                nc.sync.dma_start_transpose(out=xt2, in_=x2bf)
                ut2 = utp.tile([128, 2, 512], BF16, tag="ut", name=f"ut2{p}")
                for h in range(2):
                    u = pjp.tile([128, 512], F32, tag="pj", name=f"utps{p}{h}")
                    nc.tensor.matmul(
                        out=u, lhsT=g2, rhs=xt2[:, 4 * h : 4 * h + 4, :]
                    )
                    if h == 0:
                        nc.scalar.copy(out=ut2[:, h, :], in_=u)
                    else:
                        nc.vector.tensor_copy(out=ut2[:, h, :], in_=u)
                # V for both batches: blockdiag WvT, 128-col output per tile
                v_ps = stp.tile([128, NT, 128], F32, tag="st", name=f"vps{p}")
                for so in range(NT):
                    nc.tensor.matmul(
                        out=v_ps[:, so, :], lhsT=xt2[:, so, :], rhs=wvt2
                    )
                nc.vector.tensor_copy(
                    out=vq[:, b0 * NT : (b0 + 1) * NT, 0:64],
                    in_=v_ps[:, :, 0:64],
                )
                nc.vector.tensor_copy(
                    out=vq[:, (b0 + 1) * NT : (b0 + 2) * NT, 0:64],
                    in_=v_ps[:, :, 64:128],
                )
                return xt2, ut2

            def attn(b, i, xt2, ut2, last=False):
                """Causal attention for one batch (partition half i of pair)."""
                r0 = 64 * i
                ot_h = [
                    otp.tile([65, 512], F32, tag="ot", name=f"ot{b}{h}")
                    for h in range(2)
                ]
                for j in range(NT):
                    qa = j * 128
                    st = stp.tile([128, 1024], F32, tag="st", name=f"st{b}{j}")
                    for ca, cb in _chunks(qa, S):
                        nc.tensor.matmul(
                            out=st[:, ca:cb],
                            lhsT=xt2[r0 : r0 + 64, j, :],
                            rhs=ut2[
                                r0 : r0 + 64, ca // 512, ca % 512 : ca % 512 + cb - ca
                            ],
                        )
                    # pt local col c <-> q = qa + c
                    pt = ptp.tile([128, 1024], BF16, tag="pt", name=f"pt{b}{j}")
                    nc.scalar.activation(
                        out=pt[:, 0 : S - qa], in_=st[:, qa:S], func=Exp
                    )
                    # triangular causal mask on the diagonal block
                    nc.gpsimd.affine_select(
                        out=pt[:, 0:128],
                        in_=pt[:, 0:128],
                        compare_op=mybir.AluOpType.is_ge,
                        fill=0.0,
                        base=0,
                        pattern=[[1, 128]],
                        channel_multiplier=-1,
                    )
                    for ca, cb in _chunks(qa, S):
                        h = ca // 512
                        nc.tensor.matmul(
                            out=ot_h[h][:, ca - 512 * h : cb - 512 * h],
                            lhsT=vq[:, b * NT + j, :],
                            rhs=pt[:, ca - qa : cb - qa],
                            start=(j == 0),
                            stop=(j == 3 if h == 0 else j == 7),
                            skip_group_check=True,
                        )
                # [80, ...]: rows 65:80 stay unwritten junk; the XBAR reads
                # them but their transposed columns are never consumed
                otsb = osbp.tile([80, 1024], BF16, tag="osb", name=f"otsb{b}")
                nc.vector.tensor_copy(out=otsb[0:65, 0:512], in_=ot_h[0])
                nc.vector.tensor_copy(out=otsb[0:65, 512:1024], in_=ot_h[1])
                if not last:
                    # off-critical-path batches: XBAR DMA un-transpose
                    ot_nat = onp.tile(
                        [128, NT, 80], BF16, tag="on", name=f"onat{b}"
                    )
                    nc.sync.dma_start_transpose(out=ot_nat, in_=otsb)
                    src = ot_nat
                else:
                    # last batch: PE transposes avoid DMA-semaphore tail hops
                    # (inner dim padded to 66 to keep PSUM writes 4B-aligned)
                    ot_tp = pjp.tile([128, NT, 66], BF16, tag="pj", name="ottp")
                    for so in range(NT):
                        nc.tensor.matmul(
                            out=ot_tp[:, so, 0:65],
                            lhsT=otsb[0:65, so * 128 : (so + 1) * 128],
                            rhs=idb,
                            is_transpose=True,
                        )
                    src = ot_tp
                rsb = rpp.tile([128, NT], F32, tag="r", name=f"rsb{b}")
                nc.vector.reciprocal(out=rsb, in_=src[:, :, 64])
                r_bc = bass.AP(
                    tensor=rsb.tensor,
                    offset=rsb.offset,
                    ap=[rsb.ap[0], rsb.ap[1], [0, D]],
                )
                osb = obp.tile([128, NT, D], F32, tag="ob", name=f"osb{b}")
                nc.vector.tensor_mul(out=osb, in0=src[:, :, 0:64], in1=r_bc)
                nc.sync.dma_start(
                    out=out[b].rearrange("(so p) d -> p so d", p=128), in_=osb
                )

            xt2_0, ut2_0 = prep_pair(0)
            attn(0, 0, xt2_0, ut2_0)
            xt2_1, ut2_1 = prep_pair(1)
            attn(1, 1, xt2_0, ut2_0)
            attn(2, 0, xt2_1, ut2_1)
            attn(3, 1, xt2_1, ut2_1, last=True)
    nc.compile()
    return nc


_NC_CACHE = []
LAST_RESULTS = None


def kernel(x, Wq, Wk, Wv):
    global LAST_RESULTS
    if not _NC_CACHE:
        _NC_CACHE.append(build_bass())
    nc = _NC_CACHE[0]
    x = np.ascontiguousarray(x, dtype=np.float32)
    in_maps = [
        {
            "x": np.ascontiguousarray(x[c * B : (c + 1) * B]),
            "wq": np.ascontiguousarray(Wq, dtype=np.float32),
            "wk": np.ascontiguousarray(Wk, dtype=np.float32),
            "wv": np.ascontiguousarray(Wv, dtype=np.float32),
        }
        for c in range(N_CORES)
    ]
    res = run_bass_kernel_spmd(nc, in_maps, core_ids=list(range(N_CORES)))
    LAST_RESULTS = res
    return np.concatenate([r["out"] for r in res.results], axis=0)
